# revision 1
# baseline (speedup 1.0000x reference)
"""MemAut forward kernel for TRN2 (Bass/Tile).

Computes, per core, for z [B, 576] and memory [100, 576]:
    zn = z / ||z||; mn = memory / ||memory||
    logit = zn @ mn.T                       # [B, 100]
    w = softmax(logit, axis=1)
    w = w * (w > 1/100)                     # hard shrink (exact formula below)
    w = w / sum(|w|)                        # L1 renorm
    out = w @ memory                        # [B, 576]

Implementation notes:
  - softmax max-subtraction is skipped: logits are cosine similarities in
    [-1, 1], so exp() cannot overflow.
  - shrink+renorm is computed scale-invariantly on e = exp(l/||z||):
        keep_j = e_j > (1/100) * sum(e)   (equivalent to w_j > 1/100)
        out = (e*keep) @ memory / sum(e*keep)
    The softmax division cancels, saving ops.
  - 1/||z|| via ACT Sqrt + 2 Newton refinement steps (ACT sqrt table is
    low precision) + DVE iterative reciprocal (accurate).
"""

from contextlib import ExitStack

import concourse.bass as bass
import concourse.mybir as mybir
import concourse.tile as tile
from concourse.masks import make_identity

F32 = mybir.dt.float32
AF = mybir.ActivationFunctionType
OP = mybir.AluOpType

FEAT = 576
NMEM = 100
THRESH = 1.0 / NMEM
FCH = [128, 128, 128, 128, 64]  # feature chunks of 576


I32 = mybir.dt.int32
RSQRT_MAGIC = 0x5F3759DF


def _rsqrt_dve(nc, pool, ss, p, w, magic):
    """rnorm [p, w] = 1/sqrt(ss[p, w]), DVE-only (no ACT table switch).

    Quake-style integer seed + 3 Newton iterations -> fp32-exact.
    `magic` is a const int32 tile holding RSQRT_MAGIC.
    """
    sh = pool.tile([p, w], F32, tag="rq_sh")
    nc.vector.tensor_scalar(
        out=sh.bitcast(I32), in0=ss.bitcast(I32), scalar1=1, scalar2=None,
        op0=OP.logical_shift_right,
    )
    y = pool.tile([p, w], F32, tag="rq_y")
    nc.vector.scalar_tensor_tensor(
        out=y.bitcast(I32), in0=magic[:p, :w], scalar=0, in1=sh.bitcast(I32),
        op0=OP.add, op1=OP.subtract,
    )
    for it in range(3):
        a = pool.tile([p, w], F32, tag=f"rq_a{it}")
        nc.vector.tensor_mul(a, y, y)
        b = pool.tile([p, w], F32, tag=f"rq_b{it}")
        nc.vector.tensor_mul(b, ss, a)
        c = pool.tile([p, w], F32, tag=f"rq_c{it}")
        # c = 1.5 - 0.5*b
        nc.vector.tensor_scalar(
            out=c, in0=b, scalar1=-0.5, scalar2=1.5, op0=OP.mult, op1=OP.add
        )
        y2 = pool.tile([p, w], F32, tag=f"rq_y{it}")
        nc.vector.tensor_mul(y2, y, c)
        y = y2
    return y


def memaut_tile_kernel(ctx: ExitStack, tc: tile.TileContext, z, mem, out, reps=1):
    nc = tc.nc
    B = z.shape[0]
    RPG = 512  # rows per DMA group
    TPG = RPG // 128  # tiles per group
    assert B % RPG == 0
    ngroups = B // RPG

    singles = ctx.enter_context(tc.tile_pool(name="singles", bufs=1))

    # --- preamble: identity, memory bank prep -------------------------------
    ident = singles.tile([128, 128], F32)
    make_identity(nc, ident)
    magic = singles.tile([128, TPG], I32)
    nc.vector.memset(magic, RSQRT_MAGIC)

    mem_sb = singles.tile([NMEM, FEAT], F32)
    nc.sync.dma_start(out=mem_sb, in_=mem)

    pre = ctx.enter_context(tc.tile_pool(name="pre", bufs=1))
    msq = pre.tile([NMEM, FEAT], F32)
    mss = pre.tile([NMEM, 1], F32)
    nc.scalar.activation(out=msq, in_=mem_sb, func=AF.Square, accum_out=mss)
    rmn = _rsqrt_dve(nc, pre, mss, NMEM, 1, magic)
    mn_sb = singles.tile([NMEM, FEAT], F32)
    nc.scalar.mul(mn_sb, mem_sb, rmn)

    # mnT[:, k*100:(k+1)*100] = mn[:, 128k:128k+fw].T   (feature chunk k)
    mnT = singles.tile([128, 5 * NMEM], F32)
    with tc.tile_pool(name="pre_ps", bufs=2, space="PSUM") as pre_ps:
        for k in range(5):
            fs, fw = 128 * k, FCH[k]
            tp = pre_ps.tile([128, NMEM], F32, tag="mnT_ps")
            nc.tensor.transpose(tp[:fw, :], mn_sb[:, fs : fs + fw], ident[:NMEM, :NMEM])
            nc.vector.tensor_copy(mnT[:fw, k * NMEM : (k + 1) * NMEM], tp[:fw, :])

    # --- pools for the main loop -------------------------------------------
    zin_pool = ctx.enter_context(tc.tile_pool(name="zin", bufs=3))
    out_pool = ctx.enter_context(tc.tile_pool(name="outb", bufs=3))
    sq_pool = ctx.enter_context(tc.tile_pool(name="sq", bufs=2))
    zts_pool = ctx.enter_context(tc.tile_pool(name="zts", bufs=2))
    e_pool = ctx.enter_context(tc.tile_pool(name="e", bufs=2))
    es_pool = ctx.enter_context(tc.tile_pool(name="es", bufs=2))
    wts_pool = ctx.enter_context(tc.tile_pool(name="wts", bufs=2))
    st_pool = ctx.enter_context(tc.tile_pool(name="st", bufs=3))

    ztp_pool = ctx.enter_context(tc.tile_pool(name="ztp", bufs=2, space="PSUM"))
    zt4_pool = ctx.enter_context(tc.tile_pool(name="zt4", bufs=1, space="PSUM"))
    lg_pool = ctx.enter_context(tc.tile_pool(name="lg", bufs=2, space="PSUM"))
    wtp_pool = ctx.enter_context(tc.tile_pool(name="wtp", bufs=1, space="PSUM"))
    zh_pool = ctx.enter_context(tc.tile_pool(name="zh", bufs=1, space="PSUM"))

    zr = z.rearrange("(g n p) f -> g p n f", p=128, n=TPG)
    outr = out.rearrange("(g n p) f -> g p n f", p=128, n=TPG)

    def _load_group(g):
        zin = zin_pool.tile([128, TPG, FEAT], F32)
        nc.sync.dma_start(out=zin, in_=zr[g])
        return zin

    def _square(zin, ss_g, t):
        zsq = sq_pool.tile([128, FEAT], F32, tag="zsq")
        nc.scalar.activation(
            out=zsq, in_=zin[:, t, :], func=AF.Square,
            accum_out=ss_g[:, t : t + 1],
        )

    def _groups_body():
        # software pipeline: group g's norms are computed during group g-1's
        # tile processing, so the first exp of a group never waits on them.
        zin = _load_group(0)
        ss = st_pool.tile([128, TPG], F32, tag="ss_g")
        for t in range(TPG):
            _square(zin, ss, t)
        rnorm = _rsqrt_dve(nc, st_pool, ss, 128, TPG, magic)
        for g in range(ngroups):
            nxt = None
            if g + 1 < ngroups:
                nxt_zin = _load_group(g + 1)
                nxt_ss = st_pool.tile([128, TPG], F32, tag="ss_g")
                nxt = (nxt_zin, nxt_ss)
            _one_group(g, zin, rnorm, nxt)
            if nxt is not None:
                zin, ss = nxt
                rnorm = _rsqrt_dve(nc, st_pool, ss, 128, TPG, magic)

    def _one_group(g, zin, rnorm_g, nxt):
        out_sb = out_pool.tile([128, TPG, FEAT], F32)

        for t in range(TPG):
            zt = zin[:, t, :]

            # transpose z tile: chunks 0-3 into ztp (1 bank), chunk 4 into zt4
            ztp = ztp_pool.tile([128, 512], F32, tag="ztp")
            for k in range(4):
                nc.tensor.transpose(
                    ztp[:, k * 128 : k * 128 + 128], zt[:, 128 * k : 128 * k + 128],
                    ident,
                )
            zt4 = zt4_pool.tile([64, 128], F32, tag="zt4")
            nc.tensor.transpose(zt4, zt[:, 512:576], ident)
            zts = zts_pool.tile([128, 5 * 128], F32, tag="zts")
            nc.vector.tensor_copy(zts[:, 0:512], ztp[:, 0:512])
            nc.scalar.copy(zts[:64, 512:640], zt4)

            # logits = z @ mn.T  (accumulate over feature chunks)
            lg = lg_pool.tile([128, NMEM], F32, tag="lg")
            for k in range(5):
                fw = FCH[k]
                nc.tensor.matmul(
                    lg,
                    lhsT=zts[:fw, k * 128 : k * 128 + 128],
                    rhs=mnT[:fw, k * NMEM : (k + 1) * NMEM],
                    start=(k == 0),
                    stop=(k == 4),
                )

            # e = exp(logit / ||z||), S = sum(e)
            e = e_pool.tile([128, NMEM], F32, tag="e")
            S = st_pool.tile([128, 1], F32, tag="S")
            nc.scalar.activation(
                out=e, in_=lg, func=AF.Exp,
                scale=rnorm_g[:, t : t + 1], accum_out=S,
            )

            # es = e * (e > THRESH*S), L1 = sum(es)
            th = st_pool.tile([128, 1], F32, tag="th")
            nc.vector.tensor_scalar_mul(th, S, THRESH)
            es = es_pool.tile([128, NMEM], F32, tag="es")
            L1 = st_pool.tile([128, 1], F32, tag="L1")
            nc.vector.scalar_tensor_tensor(
                out=es, in0=e, scalar=th, in1=e,
                op0=OP.is_gt, op1=OP.mult, accum_out=L1,
            )
            rL1 = st_pool.tile([128, 1], F32, tag="rL1")
            nc.vector.reciprocal(rL1, L1)

            # wT = es.T ; zhat = (es @ memory) * (1/L1)
            wtp = wtp_pool.tile([NMEM, 128], F32, tag="wtp")
            nc.tensor.transpose(wtp, es, ident)
            wts = wts_pool.tile([NMEM, 128], F32, tag="wts")
            nc.vector.tensor_copy(wts, wtp)

            zh = zh_pool.tile([128, FEAT], F32, tag="zh")
            nc.tensor.matmul(
                zh[:, 0:512], lhsT=wts, rhs=mem_sb[:, 0:512], start=True, stop=True
            )
            nc.tensor.matmul(
                zh[:, 512:576], lhsT=wts, rhs=mem_sb[:, 512:576],
                start=True, stop=True,
            )
            nc.scalar.activation(
                out=out_sb[:, t, :], in_=zh, func=AF.Copy, scale=rL1
            )
            # prefetch-compute next group's norms in ACT idle slots
            if nxt is not None:
                _square(nxt[0], nxt[1], t)

        nc.sync.dma_start(out=outr[g], in_=out_sb)

    if reps > 1:
        with tc.For_i(0, reps, 1):
            _groups_body()
    else:
        _groups_body()



def _build(B: int):
    import concourse.bacc as bacc

    nc = bacc.Bacc("TRN2", target_bir_lowering=False, debug=False)
    z = nc.dram_tensor("z", [B, FEAT], F32, kind="ExternalInput").ap()
    mem = nc.dram_tensor("memory", [NMEM, FEAT], F32, kind="ExternalInput").ap()
    out = nc.dram_tensor("out", [B, FEAT], F32, kind="ExternalOutput").ap()
    with tile.TileContext(nc) as tc:
        with ExitStack() as ctx:
            memaut_tile_kernel(ctx, tc, z, mem, out)
    nc.compile()
    return nc


N_CORES = 8
B_TOTAL = 131072
B_CORE = B_TOTAL // N_CORES
_CACHE = {}


def kernel(z, memory):
    import numpy as np
    from concourse.bass_utils import run_bass_kernel_spmd

    z = np.ascontiguousarray(z, dtype=np.float32)
    memory = np.ascontiguousarray(memory, dtype=np.float32)
    assert z.shape == (B_TOTAL, FEAT) and memory.shape == (NMEM, FEAT)

    if "nc" not in _CACHE:
        _CACHE["nc"] = _build(B_CORE)
    nc = _CACHE["nc"]

    shards = z.reshape(N_CORES, B_CORE, FEAT)
    in_maps = [{"z": shards[i], "memory": memory} for i in range(N_CORES)]
    res = run_bass_kernel_spmd(nc, in_maps, core_ids=list(range(N_CORES)))
    out = np.concatenate([res.results[i]["out"] for i in range(N_CORES)], axis=0)
    return out.astype(np.float32, copy=False)



# revision 3
# speedup vs baseline: 52.0282x; 52.0282x over previous
"""MemAut forward kernel v2 for TRN2 (Bass/Tile).

Key design points vs v1:
  - fp16 compute: z cast to fp16 on host (halves HBM traffic + 1cyc/row PE);
    output written fp16, cast back to fp32 on host. Validated mean rel err
    ~2.3e-4 vs fp64 reference (gate is 2e-2).
  - e = exp(logit/||z||) kept in fp32 through the hard-shrink compare to
    avoid threshold flips; es cast fp16 only for the read-out matmul.
  - Hardware For_i over group blocks: the unrolled 32-group body (~3264
    instructions) thrashes IRAM (16 KiB blocks; >256 instr/engine re-fetches
    from HBM every rep => measured ~9 us/instr). Body = U groups keeps every
    engine under 256 instructions.
  - Engine balance: squares+shrink on DVE, exp+sqrt+half the out-copies on
    ACT, PSUM->SBUF copies split DVE/Pool, out-copies split ACT/Pool.
"""

from contextlib import ExitStack

import concourse.bass as bass
import concourse.mybir as mybir
import concourse.tile as tile
from concourse.masks import make_identity

F32 = mybir.dt.float32
F16 = mybir.dt.float16
AF = mybir.ActivationFunctionType
OP = mybir.AluOpType

FEAT = 576
NMEM = 100
THRESH = 1.0 / NMEM
FCH = [128, 128, 128, 128, 64]


def memaut_v2(ctx: ExitStack, tc: tile.TileContext, z, mem, out, reps=1, U=4,
              staggered=False, hints=()):
    nc = tc.nc
    B = z.shape[0]
    RPG = 512
    TPG = RPG // 128
    assert B % (U * RPG) == 0
    outer = B // (U * RPG)

    singles = ctx.enter_context(tc.tile_pool(name="singles", bufs=1))
    ident16 = singles.tile([128, 128], F16)
    make_identity(nc, ident16)

    # ---- memory bank prep (fp32 in, fp16 normalized + raw copies) ----------
    mem_sb = singles.tile([NMEM, FEAT], F32)
    nc.sync.dma_start(out=mem_sb, in_=mem)

    pre = ctx.enter_context(tc.tile_pool(name="pre", bufs=1))
    msq = pre.tile([NMEM, FEAT], F32)
    mss = pre.tile([NMEM, 1], F32)
    nc.scalar.activation(out=msq, in_=mem_sb, func=AF.Square, accum_out=mss)
    msqr = pre.tile([NMEM, 1], F32)
    nc.scalar.activation(out=msqr, in_=mss, func=AF.Sqrt)
    my0 = pre.tile([NMEM, 1], F32)
    nc.vector.reciprocal(my0, msqr)
    # one Newton step on rsqrt: y1 = y0*(1.5 - 0.5*ss*y0^2)
    ma = pre.tile([NMEM, 1], F32)
    nc.vector.tensor_mul(ma, my0, my0)
    mb = pre.tile([NMEM, 1], F32)
    nc.vector.tensor_mul(mb, mss, ma)
    mc = pre.tile([NMEM, 1], F32)
    nc.vector.tensor_scalar(out=mc, in0=mb, scalar1=-0.5, scalar2=1.5,
                            op0=OP.mult, op1=OP.add)
    rmn = pre.tile([NMEM, 1], F32)
    nc.vector.tensor_mul(rmn, my0, mc)

    mn16 = singles.tile([NMEM, FEAT], F16)
    nc.scalar.activation(out=mn16, in_=mem_sb, func=AF.Copy, scale=rmn)
    mem16 = singles.tile([NMEM, FEAT], F16)
    nc.scalar.copy(mem16, mem_sb)

    # mnT[:, k*100:(k+1)*100] = mn16[:, 128k:128k+fw].T
    mnT = singles.tile([128, 5 * NMEM], F16)
    with tc.tile_pool(name="pre_ps", bufs=1, space="PSUM") as pre_ps:
        mtp = pre_ps.tile([128, 5 * NMEM], F16)
        for k in range(5):
            fs, fw = 128 * k, FCH[k]
            nc.tensor.transpose(
                mtp[:fw, k * NMEM:(k + 1) * NMEM],
                mn16[:, fs:fs + fw], ident16[:NMEM, :NMEM])
        nc.vector.tensor_copy(mnT, mtp)

    # ---- pools -------------------------------------------------------------
    zin_pool = ctx.enter_context(tc.tile_pool(name="zin", bufs=3))
    out_pool = ctx.enter_context(tc.tile_pool(name="outb", bufs=3))
    zsq_pool = ctx.enter_context(tc.tile_pool(name="zsq", bufs=2))
    zts_pool = ctx.enter_context(tc.tile_pool(name="zts", bufs=2))
    e_pool = ctx.enter_context(tc.tile_pool(name="e", bufs=2))
    es_pool = ctx.enter_context(tc.tile_pool(name="es", bufs=2))
    wts_pool = ctx.enter_context(tc.tile_pool(name="wts", bufs=2))
    st_pool = ctx.enter_context(tc.tile_pool(name="st", bufs=3))

    ztp_pool = ctx.enter_context(tc.tile_pool(name="ztp", bufs=2, space="PSUM"))
    lg_pool = ctx.enter_context(tc.tile_pool(name="lg", bufs=2, space="PSUM"))
    wtp_pool = ctx.enter_context(tc.tile_pool(name="wtp", bufs=2, space="PSUM"))
    zh_pool = ctx.enter_context(tc.tile_pool(name="zh", bufs=1, space="PSUM"))

    zr = z.rearrange("(o u n p) f -> o u p n f", u=U, n=TPG, p=128)
    outr = out.rearrange("(o u n p) f -> o u p n f", u=U, n=TPG, p=128)

    def one_group(i, u):
        zin = zin_pool.tile([128, TPG, FEAT], F16)
        nc.sync.dma_start(out=zin, in_=zr[i][u])

        # row norms: sumsq (DVE) -> sqrt (ACT) -> recip + Newton (DVE)
        ss = st_pool.tile([128, TPG], F32, tag="ss")
        for t in range(TPG):
            zsq = zsq_pool.tile([128, FEAT], F16, tag="zsq")
            nc.scalar.activation(
                out=zsq, in_=zin[:, t, :], func=AF.Square,
                accum_out=ss[:, t:t + 1])
        sqs = st_pool.tile([128, TPG], F32, tag="sqs")
        nc.scalar.activation(out=sqs, in_=ss, func=AF.Sqrt)
        y0 = st_pool.tile([128, TPG], F32, tag="y0")
        nc.vector.reciprocal(y0, sqs)
        na = st_pool.tile([128, TPG], F32, tag="na")
        nc.gpsimd.tensor_mul(na, y0, y0)
        nb = st_pool.tile([128, TPG], F32, tag="nb")
        nc.gpsimd.tensor_mul(nb, ss, na)
        ncc = st_pool.tile([128, TPG], F32, tag="ncc")
        nc.gpsimd.tensor_scalar(out=ncc, in0=nb, scalar1=-0.5, scalar2=1.5,
                                op0=OP.mult, op1=OP.add)
        rnorm = st_pool.tile([128, TPG], F32, tag="rnorm")
        nc.gpsimd.tensor_mul(rnorm, y0, ncc)

        out_sb = out_pool.tile([128, TPG, FEAT], F16)
        for t in range(TPG):
            zt = zin[:, t, :]

            # transpose z tile into one PSUM bank: 4x128 chunks + 64 chunk
            ztp = ztp_pool.tile([128, 640], F16, tag="ztp")
            for k in range(4):
                nc.tensor.transpose(
                    ztp[:, k * 128:k * 128 + 128], zt[:, 128 * k:128 * k + 128],
                    ident16)
            nc.tensor.transpose(ztp[:64, 512:640], zt[:, 512:576], ident16)
            zts = zts_pool.tile([128, 640], F16, tag="zts")
            if t % 2 == 0:
                nc.vector.tensor_copy(zts, ztp)
            else:
                nc.scalar.copy(zts, ztp)

            # logits = z @ mn.T (fp16 inputs, fp32 PSUM accum)
            lg = lg_pool.tile([128, NMEM], F32, tag="lg")
            for k in range(5):
                fw = FCH[k]
                nc.tensor.matmul(
                    lg, lhsT=zts[:fw, k * 128:k * 128 + 128],
                    rhs=mnT[:fw, k * NMEM:(k + 1) * NMEM],
                    start=(k == 0), stop=(k == 4))

            # e = exp(logit/||z||) in fp32, S = sum(e)
            e = e_pool.tile([128, NMEM], F32, tag="e")
            S = st_pool.tile([128, 1], F32, tag="S")
            nc.scalar.activation(
                out=e, in_=lg, func=AF.Exp,
                scale=rnorm[:, t:t + 1], accum_out=S)

            # es = e*(e > S/100) in fp16, L1 = sum(es)
            th = st_pool.tile([128, 1], F32, tag="th")
            nc.gpsimd.tensor_scalar_mul(th, S, THRESH)
            es = es_pool.tile([128, NMEM], F16, tag="es")
            L1 = st_pool.tile([128, 1], F32, tag="L1")
            nc.vector.scalar_tensor_tensor(
                out=es, in0=e, scalar=th, in1=e,
                op0=OP.is_gt, op1=OP.mult, accum_out=L1)
            rL1 = st_pool.tile([128, 1], F32, tag="rL1")
            nc.vector.reciprocal(rL1, L1)

            # esT, zhat = (es @ mem) * (1/L1)
            wtp = wtp_pool.tile([NMEM, 128], F16, tag="wtp")
            nc.tensor.transpose(wtp, es, ident16)
            wts = wts_pool.tile([NMEM, 128], F16, tag="wts")
            nc.vector.tensor_copy(wts, wtp)

            zh = zh_pool.tile([128, FEAT], F32, tag="zh")
            nc.tensor.matmul(zh[:, 0:512], lhsT=wts, rhs=mem16[:, 0:512],
                             start=True, stop=True)
            nc.tensor.matmul(zh[:, 512:576], lhsT=wts, rhs=mem16[:, 512:576],
                             start=True, stop=True)
            if t == 1:
                nc.vector.tensor_scalar_mul(out_sb[:, t, :], zh, rL1)
            else:
                nc.scalar.activation(
                    out=out_sb[:, t, :], in_=zh, func=AF.Copy, scale=rL1)

        nc.sync.dma_start(out=outr[i][u], in_=out_sb)

    def body(i):
        for u in range(U):
            one_group(i, u)

    def groups(i):
        if outer > 1:
            with tc.For_i(0, outer, 1, staggered_reset=staggered,
                          hint_engines=hints) as g:
                body(g)
        else:
            body(0)

    if reps > 1:
        with tc.For_i(0, reps, 1):
            groups(None)
    else:
        groups(None)


def build(B: int, reps: int = 1, U: int = 4, staggered: bool = False,
          hints: tuple = ()):
    import concourse.bacc as bacc

    nc = bacc.Bacc("TRN2", target_bir_lowering=False, debug=False)
    z = nc.dram_tensor("z", [B, FEAT], F16, kind="ExternalInput").ap()
    mem = nc.dram_tensor("memory", [NMEM, FEAT], F32, kind="ExternalInput").ap()
    out = nc.dram_tensor("out", [B, FEAT], F16, kind="ExternalOutput").ap()
    with tile.TileContext(nc) as tc:
        with ExitStack() as ctx:
            memaut_v2(ctx, tc, z, mem, out, reps=reps, U=U, staggered=staggered,
                      hints=hints)
    nc.compile()
    return nc


N_CORES = 8
B_TOTAL = 131072
B_CORE = B_TOTAL // N_CORES
_CACHE = {}


def kernel(z, memory):
    import numpy as np
    from concourse.bass_utils import run_bass_kernel_spmd

    z = np.ascontiguousarray(z, dtype=np.float32)
    memory = np.ascontiguousarray(memory, dtype=np.float32)
    assert z.shape == (B_TOTAL, FEAT) and memory.shape == (NMEM, FEAT)

    if "nc" not in _CACHE:
        _CACHE["nc"] = build(B_CORE)
    nc = _CACHE["nc"]

    z16 = z.astype(np.float16)
    shards = z16.reshape(N_CORES, B_CORE, FEAT)
    in_maps = [{"z": shards[i], "memory": memory} for i in range(N_CORES)]
    res = run_bass_kernel_spmd(nc, in_maps, core_ids=list(range(N_CORES)))
    out = np.concatenate([res.results[i]["out"] for i in range(N_CORES)], axis=0)
    return out.astype(np.float32)




# revision 6
# speedup vs baseline: 57.8306x; 1.1115x over previous
"""MemAut forward kernel v2 for TRN2 (Bass/Tile).

Key design points vs v1:
  - fp16 compute: z cast to fp16 on host (halves HBM traffic + 1cyc/row PE);
    output written fp16, cast back to fp32 on host. Validated mean rel err
    ~2.3e-4 vs fp64 reference (gate is 2e-2).
  - e = exp(logit/||z||) kept in fp32 through the hard-shrink compare to
    avoid threshold flips; es cast fp16 only for the read-out matmul.
  - Hardware For_i over group blocks: the unrolled 32-group body (~3264
    instructions) thrashes IRAM (16 KiB blocks; >256 instr/engine re-fetches
    from HBM every rep => measured ~9 us/instr). Body = U groups keeps every
    engine under 256 instructions.
  - Engine balance: squares+shrink on DVE, exp+sqrt+half the out-copies on
    ACT, PSUM->SBUF copies split DVE/Pool, out-copies split ACT/Pool.
"""

from contextlib import ExitStack

import concourse.bass as bass
import concourse.mybir as mybir
import concourse.tile as tile
from concourse.masks import make_identity

F32 = mybir.dt.float32
F16 = mybir.dt.float16
AF = mybir.ActivationFunctionType
OP = mybir.AluOpType

FEAT = 576
NMEM = 100
THRESH = 1.0 / NMEM
FCH = [128, 128, 128, 128, 64]


def memaut_v2(ctx: ExitStack, tc: tile.TileContext, z, mem, out, reps=1, U=4,
              staggered=False, hints=()):
    nc = tc.nc
    B = z.shape[0]
    RPG = 512
    TPG = RPG // 128
    assert B % (U * RPG) == 0
    outer = B // (U * RPG)

    singles = ctx.enter_context(tc.tile_pool(name="singles", bufs=1))
    ident16 = singles.tile([128, 128], F16)
    make_identity(nc, ident16)

    # ---- memory bank prep (fp32 in, fp16 normalized + raw copies) ----------
    mem_sb = singles.tile([NMEM, FEAT], F32)
    nc.sync.dma_start(out=mem_sb, in_=mem)

    pre = ctx.enter_context(tc.tile_pool(name="pre", bufs=1))
    msq = pre.tile([NMEM, FEAT], F32)
    mss = pre.tile([NMEM, 1], F32)
    nc.scalar.activation(out=msq, in_=mem_sb, func=AF.Square, accum_out=mss)
    msqr = pre.tile([NMEM, 1], F32)
    nc.scalar.activation(out=msqr, in_=mss, func=AF.Sqrt)
    my0 = pre.tile([NMEM, 1], F32)
    nc.vector.reciprocal(my0, msqr)
    # one Newton step on rsqrt: y1 = y0*(1.5 - 0.5*ss*y0^2)
    ma = pre.tile([NMEM, 1], F32)
    nc.vector.tensor_mul(ma, my0, my0)
    mb = pre.tile([NMEM, 1], F32)
    nc.vector.tensor_mul(mb, mss, ma)
    mc = pre.tile([NMEM, 1], F32)
    nc.vector.tensor_scalar(out=mc, in0=mb, scalar1=-0.5, scalar2=1.5,
                            op0=OP.mult, op1=OP.add)
    rmn = pre.tile([NMEM, 1], F32)
    nc.vector.tensor_mul(rmn, my0, mc)

    mn16 = singles.tile([NMEM, FEAT], F16)
    nc.scalar.activation(out=mn16, in_=mem_sb, func=AF.Copy, scale=rmn)
    mem16 = singles.tile([NMEM, FEAT], F16)
    nc.scalar.copy(mem16, mem_sb)

    # mnT[:, k*100:(k+1)*100] = mn16[:, 128k:128k+fw].T
    mnT = singles.tile([128, 5 * NMEM], F16)
    with tc.tile_pool(name="pre_ps", bufs=1, space="PSUM") as pre_ps:
        mtp = pre_ps.tile([128, 5 * NMEM], F16)
        for k in range(5):
            fs, fw = 128 * k, FCH[k]
            nc.tensor.transpose(
                mtp[:fw, k * NMEM:(k + 1) * NMEM],
                mn16[:, fs:fs + fw], ident16[:NMEM, :NMEM])
        nc.vector.tensor_copy(mnT, mtp)

    # ---- pools -------------------------------------------------------------
    zin_pool = ctx.enter_context(tc.tile_pool(name="zin", bufs=3))
    out_pool = ctx.enter_context(tc.tile_pool(name="outb", bufs=3))
    zsq_pool = ctx.enter_context(tc.tile_pool(name="zsq", bufs=2))
    zts_pool = ctx.enter_context(tc.tile_pool(name="zts", bufs=2))
    e_pool = ctx.enter_context(tc.tile_pool(name="e", bufs=2))
    es_pool = ctx.enter_context(tc.tile_pool(name="es", bufs=2))
    wts_pool = ctx.enter_context(tc.tile_pool(name="wts", bufs=2))
    st_pool = ctx.enter_context(tc.tile_pool(name="st", bufs=3))

    ztp_pool = ctx.enter_context(tc.tile_pool(name="ztp", bufs=2, space="PSUM"))
    lg_pool = ctx.enter_context(tc.tile_pool(name="lg", bufs=2, space="PSUM"))
    wtp_pool = ctx.enter_context(tc.tile_pool(name="wtp", bufs=2, space="PSUM"))
    zh_pool = ctx.enter_context(tc.tile_pool(name="zh", bufs=1, space="PSUM"))

    zr = z.rearrange("(o u n p) f -> o u p n f", u=U, n=TPG, p=128)
    outr = out.rearrange("(o u n p) f -> o u p n f", u=U, n=TPG, p=128)

    def one_group(i, u):
        zin = zin_pool.tile([128, TPG, FEAT], F16)
        nc.sync.dma_start(out=zin, in_=zr[i][u])

        # row norms: sumsq (DVE) -> sqrt (ACT) -> recip + Newton (DVE)
        ss = st_pool.tile([128, TPG], F32, tag="ss")
        for t in range(TPG):
            zsq = zsq_pool.tile([128, FEAT], F16, tag="zsq")
            nc.scalar.activation(
                out=zsq, in_=zin[:, t, :], func=AF.Square,
                accum_out=ss[:, t:t + 1])
        sqs = st_pool.tile([128, TPG], F32, tag="sqs")
        nc.scalar.activation(out=sqs, in_=ss, func=AF.Sqrt)
        y0 = st_pool.tile([128, TPG], F32, tag="y0")
        nc.vector.reciprocal(y0, sqs)
        na = st_pool.tile([128, TPG], F32, tag="na")
        nc.gpsimd.tensor_mul(na, y0, y0)
        nb = st_pool.tile([128, TPG], F32, tag="nb")
        nc.gpsimd.tensor_mul(nb, ss, na)
        ncc = st_pool.tile([128, TPG], F32, tag="ncc")
        nc.gpsimd.tensor_scalar(out=ncc, in0=nb, scalar1=-0.5, scalar2=1.5,
                                op0=OP.mult, op1=OP.add)
        rnorm = st_pool.tile([128, TPG], F32, tag="rnorm")
        nc.gpsimd.tensor_mul(rnorm, y0, ncc)

        out_sb = out_pool.tile([128, TPG, FEAT], F16)
        for t in range(TPG):
            zt = zin[:, t, :]

            # transpose z tile into one PSUM bank: 4x128 chunks + 64 chunk
            ztp = ztp_pool.tile([128, 640], F16, tag="ztp")
            for k in range(4):
                nc.tensor.transpose(
                    ztp[:, k * 128:k * 128 + 128], zt[:, 128 * k:128 * k + 128],
                    ident16)
            nc.tensor.transpose(ztp[:64, 512:640], zt[:, 512:576], ident16)
            zts = zts_pool.tile([128, 640], F16, tag="zts")
            if t % 2 == 0:
                nc.vector.tensor_copy(zts, ztp)
            else:
                nc.scalar.copy(zts, ztp)

            # logits = z @ mn.T (fp16 inputs, fp32 PSUM accum)
            lg = lg_pool.tile([128, NMEM], F32, tag="lg")
            for k in range(5):
                fw = FCH[k]
                nc.tensor.matmul(
                    lg, lhsT=zts[:fw, k * 128:k * 128 + 128],
                    rhs=mnT[:fw, k * NMEM:(k + 1) * NMEM],
                    start=(k == 0), stop=(k == 4))

            # e = exp(logit/||z||) in fp32, S = sum(e)
            e = e_pool.tile([128, NMEM], F32, tag="e")
            S = st_pool.tile([128, 1], F32, tag="S")
            nc.scalar.activation(
                out=e, in_=lg, func=AF.Exp,
                scale=rnorm[:, t:t + 1], accum_out=S)

            # es = e*(e > S/100) in fp16, L1 = sum(es)
            th = st_pool.tile([128, 1], F32, tag="th")
            nc.gpsimd.tensor_scalar_mul(th, S, THRESH)
            es = es_pool.tile([128, NMEM], F16, tag="es")
            L1 = st_pool.tile([128, 1], F32, tag="L1")
            nc.vector.scalar_tensor_tensor(
                out=es, in0=e, scalar=th, in1=e,
                op0=OP.is_gt, op1=OP.mult, accum_out=L1)
            rL1 = st_pool.tile([128, 1], F32, tag="rL1")
            nc.vector.reciprocal(rL1, L1)

            # esT, zhat = (es @ mem) * (1/L1)
            wtp = wtp_pool.tile([NMEM, 128], F16, tag="wtp")
            nc.tensor.transpose(wtp, es, ident16)
            wts = wts_pool.tile([NMEM, 128], F16, tag="wts")
            nc.vector.tensor_copy(wts, wtp)

            zh = zh_pool.tile([128, FEAT], F32, tag="zh")
            nc.tensor.matmul(zh[:, 0:512], lhsT=wts, rhs=mem16[:, 0:512],
                             start=True, stop=True)
            nc.tensor.matmul(zh[:, 512:576], lhsT=wts, rhs=mem16[:, 512:576],
                             start=True, stop=True)
            if t == 1:
                nc.vector.tensor_scalar_mul(out_sb[:, t, :], zh, rL1)
            else:
                nc.scalar.activation(
                    out=out_sb[:, t, :], in_=zh, func=AF.Copy, scale=rL1)

        nc.sync.dma_start(out=outr[i][u], in_=out_sb)

    def body(i):
        for u in range(U):
            one_group(i, u)

    def groups(i):
        if outer > 1:
            with tc.For_i(0, outer, 1, staggered_reset=staggered,
                          hint_engines=hints) as g:
                body(g)
        else:
            body(0)

    if reps > 1:
        with tc.For_i(0, reps, 1):
            groups(None)
    else:
        groups(None)


def build(B: int, reps: int = 1, U: int = 8, staggered: bool = False,
          hints: tuple = (mybir.EngineType.PE, mybir.EngineType.DVE,
                          mybir.EngineType.Activation, mybir.EngineType.Pool,
                          mybir.EngineType.SP)):
    import concourse.bacc as bacc

    nc = bacc.Bacc("TRN2", target_bir_lowering=False, debug=False)
    z = nc.dram_tensor("z", [B, FEAT], F16, kind="ExternalInput").ap()
    mem = nc.dram_tensor("memory", [NMEM, FEAT], F32, kind="ExternalInput").ap()
    out = nc.dram_tensor("out", [B, FEAT], F16, kind="ExternalOutput").ap()
    with tile.TileContext(nc) as tc:
        with ExitStack() as ctx:
            memaut_v2(ctx, tc, z, mem, out, reps=reps, U=U, staggered=staggered,
                      hints=hints)
    nc.compile()
    return nc


N_CORES = 8
B_TOTAL = 131072
B_CORE = B_TOTAL // N_CORES
_CACHE = {}


def kernel(z, memory):
    import numpy as np
    from concourse.bass_utils import run_bass_kernel_spmd

    z = np.ascontiguousarray(z, dtype=np.float32)
    memory = np.ascontiguousarray(memory, dtype=np.float32)
    assert z.shape == (B_TOTAL, FEAT) and memory.shape == (NMEM, FEAT)

    if "nc" not in _CACHE:
        _CACHE["nc"] = build(B_CORE)
    nc = _CACHE["nc"]

    z16 = z.astype(np.float16)
    shards = z16.reshape(N_CORES, B_CORE, FEAT)
    in_maps = [{"z": shards[i], "memory": memory} for i in range(N_CORES)]
    res = run_bass_kernel_spmd(nc, in_maps, core_ids=list(range(N_CORES)))
    out = np.concatenate([res.results[i]["out"] for i in range(N_CORES)], axis=0)
    return out.astype(np.float32)


